# revision 1
# baseline (speedup 1.0000x reference)
"""Trainium2 Bass kernel for nn_ChannelAttention (B=16, C=256, T=2048, L=5).

Data-parallel over 8 NeuronCores: each core processes 2 batches.

Math (per batch b, all on-device except tiny weight folding on host):
  qsum[l,t]   = sum_i (q_w[l,i,:] @ x[:,t] + q_b[l,i])  -> qws[l] @ x + qbs[l]
  scores[c,t] = sum_l (k_w[l] @ (x * qsum[l]))[c, t-l]  + sum_l k_b[l,c]*qsum[l,t-l]
  w = softmax_c(scores);  v = PReLU(BN(v_w @ x + v_b));  out = w * v

Key implementation points:
  - q-path collapsed on host: only qws=[L,C] needed (sum over output chans).
  - BN folded into v_w / v_b on host.
  - scores computed as sum_l kwT[l].T @ (x . Bq_l) with the lag handled by
    column-offset slicing of gated tiles (G columns), zero-padded at t<l.
  - Bq_l (qsum row broadcast over 128 partitions) via K=1 PE matmul with ones.
  - shifted qsum (for the k_b bias term) via a DRAM round trip with a
    partition-stride (pitch-1) flat access pattern.
  - softmax over channels: pair-max of the two 128-partition halves (DVE) +
    gpsimd.partition_all_reduce(max) which also broadcasts; exp on ACT;
    sum over channels via ones-column PE matmul; reciprocal on DVE;
    gpsimd.partition_broadcast.
  - PE matmuls run in float32r (4-byte data, reduced-precision multiply,
    1 cycle/row at N>=256) - set MM_DT to float32 for exact-but-4x-slower.
"""

import sys

sys.path.insert(0, "/opt/trn_rl_repo")

import numpy as np

import concourse.bass as bass
import concourse.mybir as mybir
import concourse.tile as tile
from concourse import bacc
from concourse import bass_isa
from concourse.bass_utils import run_bass_kernel_spmd

B, C, T, L = 16, 256, 2048, 5
NCORES = 8
BPC = B // NCORES      # batches per core
P = 128                # partitions
KC = C // P            # k chunks (2)
MC = C // P            # m chunks (2)
NT = 512               # time tile
NCHUNK = T // NT       # 4
QPITCH = T + 16        # dram scratch row pitch for shifted qsum
BN_EPS = 1e-5

F32 = mybir.dt.float32
F32R = mybir.dt.float32r
MM_DT = F32R           # matmul dtype (float32r fast / float32 exact)
# Explicit PReLU: out = a*z + b*|z| with z = w*(v+vb), a,b folded on host.
# Needed because CoreSim lacks Lrelu; also the safe fallback if HW Lrelu
# numerics are off.  False = fused ACT Lrelu (cheaper).
PRELU_EXPLICIT = False

AF = mybir.ActivationFunctionType
ALU = mybir.AluOpType


MF = MM_DT               # dtype for every tile that feeds a matmul


def _r(ap):
    """Bitcast an AP to the matmul dtype (no-op when already MF)."""
    return ap.bitcast(MM_DT) if ap.dtype is not MM_DT else ap


def _f(ap):
    """Bitcast an MF AP back to plain fp32 for non-matmul engines."""
    return ap.bitcast(F32) if ap.dtype is not F32 else ap


def build_program(alpha: float) -> bass.Bass:
    nc = bacc.Bacc("TRN2", target_bir_lowering=False, debug=False, num_devices=NCORES)

    x_in = nc.dram_tensor("x", [BPC, KC, P, T], MF, kind="ExternalInput").ap()
    kwT_in = nc.dram_tensor("kwT", [P, L, KC, MC, P], MF, kind="ExternalInput").ap()
    kb_in = nc.dram_tensor("kb", [L, MC, P], MF, kind="ExternalInput").ap()
    qwsT_in = nc.dram_tensor("qwsT", [P, KC, L], MF, kind="ExternalInput").ap()
    qbs_in = nc.dram_tensor("qbs", [L, 1], F32, kind="ExternalInput").ap()
    vwT_in = nc.dram_tensor("vwT", [P, KC, MC, P], MF, kind="ExternalInput").ap()
    vb_in = nc.dram_tensor("vb", [P, MC], F32, kind="ExternalInput").ap()
    ident_in = nc.dram_tensor("ident", [P, P], F32, kind="ExternalInput").ap()
    ones65_in = nc.dram_tensor("ones65", [65, P], MF, kind="ExternalInput").ap()
    onesc_in = nc.dram_tensor("ones_col", [P, 1], MF, kind="ExternalInput").ap()
    zeros8_in = nc.dram_tensor("zeros8", [P, KC, 8], MF, kind="ExternalInput").ap()
    y_out = nc.dram_tensor("y", [BPC, MC, P, T], F32, kind="ExternalOutput").ap()
    # scratch for the lag-shift of qsum rows (row l shifted right by l)
    qsd = nc.dram_tensor("qs_scratch", [BPC, L, QPITCH], MF).ap()

    from contextlib import ExitStack

    with tile.TileContext(nc) as tc:
        with ExitStack() as ctx:
            ep = ctx.enter_context
            ep(nc.allow_low_precision(
                reason="float32r outputs carry full fp32 bits; rounding "
                       "happens inside the PE only"
            ))
            consts = ep(tc.tile_pool(name="consts", bufs=1))
            xpool = ep(tc.tile_pool(name="xpool", bufs=2))
            qspool = ep(tc.tile_pool(name="qspool", bufs=1))
            qrowpool = ep(tc.tile_pool(name="qrowpool", bufs=1))
            wpool = ep(tc.tile_pool(name="wpool", bufs=16))
            bqsb_pool = ep(tc.tile_pool(name="bqsb", bufs=2))
            einpool = ep(tc.tile_pool(name="einpool", bufs=4))
            epool = ep(tc.tile_pool(name="epool", bufs=4))
            wspool = ep(tc.tile_pool(name="wspool", bufs=2))
            mpool = ep(tc.tile_pool(name="mpool", bufs=2))
            rpool = ep(tc.tile_pool(name="rpool", bufs=2))
            vpool = ep(tc.tile_pool(name="vpool", bufs=3))
            opool = ep(tc.tile_pool(name="opool", bufs=4))
            # PSUM: 8 banks total.  aux {qs, bq, sum} share 3; pscore 4; v 1.
            paux = ep(tc.tile_pool(name="paux", bufs=3, space="PSUM"))
            pscore_pool = ep(tc.tile_pool(name="pscore", bufs=4, space="PSUM"))
            pv_pool = ep(tc.tile_pool(name="pv", bufs=1, space="PSUM"))
            # ---- constants ----
            kwT = consts.tile([P, L, KC, MC, P], MF)
            nc.sync.dma_start(out=kwT, in_=kwT_in)
            kb = consts.tile([L, MC, P], MF)
            nc.sync.dma_start(out=kb, in_=kb_in)
            qwsT = consts.tile([P, KC, L], MF)
            nc.sync.dma_start(out=qwsT, in_=qwsT_in)
            qbs = consts.tile([L, 1], F32)
            nc.sync.dma_start(out=qbs, in_=qbs_in)
            vwT = consts.tile([P, KC, MC, P], MF)
            nc.sync.dma_start(out=vwT, in_=vwT_in)
            vb = consts.tile([P, MC], F32)
            nc.sync.dma_start(out=vb, in_=vb_in)
            ones65 = consts.tile([65, P], MF)       # lhsT rows for bcast mm
            nc.sync.dma_start(out=ones65, in_=ones65_in)
            ones_col = consts.tile([P, 1], MF)      # lhsT for channel-sum mm
            nc.sync.dma_start(out=ones_col, in_=onesc_in)
            ident = consts.tile([P, P], F32)         # for PE transpose
            nc.sync.dma_start(out=ident, in_=ident_in)
            zpad = consts.tile([L, 16], F32)         # zero left pad for qsd
            nc.vector.memset(zpad, 0.0)

            for b in range(BPC):
                # ---- load x ----
                x_sb = xpool.tile([P, KC, 8 + T], MF)
                for kc in range(KC):
                    nc.sync.dma_start(out=x_sb[:, kc, 0:8], in_=zeros8_in[:, kc, :])
                    nc.sync.dma_start(out=x_sb[:, kc, 8:8 + T], in_=x_in[b, kc])

                # ---- qsum rows: qs[l,t] = qws[l] @ x[:,t] + qbs[l] ----
                qs_sb = qspool.tile([L, T], MF, tag="qs")
                for n in range(NCHUNK):
                    qs_ps = paux.tile([L, NT], F32, tag="paux")
                    for kc in range(KC):
                        nc.tensor.matmul(
                            qs_ps,
                            _r(qwsT[:, kc, :]),
                            _r(x_sb[:, kc, 8 + n * NT:8 + (n + 1) * NT]),
                            start=(kc == 0),
                            stop=(kc == KC - 1),
                        )
                    nc.scalar.activation(
                        out=qs_sb[:, n * NT:(n + 1) * NT], in_=qs_ps,
                        func=AF.Identity, bias=qbs, scale=1.0,
                    )

                # ---- shifted qsum via DRAM round trip ----
                nc.sync.dma_start(out=qsd[b, :, 8:8 + T], in_=qs_sb)
                nc.sync.dma_start(out=qsd[b, :, 0:8], in_=_r(zpad[:, 0:8]))
                qssh_sb = qspool.tile([L, T], MF, tag="qssh")
                shifted = bass.AP(
                    tensor=qsd.tensor,
                    offset=b * L * QPITCH + 8,
                    ap=[[QPITCH - 1, L], [1, T]],
                )
                nc.sync.dma_start(out=qssh_sb, in_=shifted)
                # unshifted qsum rows at base partitions 0/32/64 (matmul
                # operands must start at partition 0, 32 or 64)
                qrowA = qrowpool.tile([65, 8 + T], MF, tag="qrA")
                qrowB = qrowpool.tile([33, 8 + T], MF, tag="qrB")
                for l in range(L):
                    rt, bp = (qrowA, 32 * l) if l < 3 else (qrowB, 32 * (l - 3))
                    nc.sync.dma_start(
                        out=rt[bp:bp + 1, :], in_=qsd[b, l:l + 1, 0:8 + T]
                    )

                # ---- time-chunk loop ----
                for n in range(NCHUNK):
                    t0 = n * NT

                    # v path: v = PReLU(vw' @ x + vb')
                    v_sbs = []
                    for mc in range(MC):
                        v_ps = pv_pool.tile([P, NT], F32, tag="pv")
                        for kc in range(KC):
                            nc.tensor.matmul(
                                v_ps,
                                _r(vwT[:, kc, mc, :]),
                                _r(x_sb[:, kc, 8 + t0:8 + t0 + NT]),
                                start=(kc == 0),
                                stop=(kc == KC - 1),
                            )
                        v_sb = vpool.tile([P, NT], F32, tag="v")
                        if PRELU_EXPLICIT:
                            nc.scalar.activation(
                                out=v_sb, in_=v_ps, func=AF.Identity,
                                bias=vb[:, mc:mc + 1], scale=1.0,
                            )
                        else:
                            nc.scalar.activation(
                                out=v_sb, in_=v_ps, func=AF.Prelu,
                                bias=vb[:, mc:mc + 1], scale=1.0, alpha=alpha,
                            )
                        v_sbs.append(v_sb)

                    # gated tiles: w_l[:, i] = x[:, t0-l+i] * qs[l, t0-l+i]
                    w_tiles = []
                    for l in range(L):
                        s0 = 8 + t0 - l              # padded source start
                        rt, bp = (qrowA, 32 * l) if l < 3 else (qrowB, 32 * (l - 3))
                        bq_ps = paux.tile([P, NT], F32, tag="paux")
                        nc.tensor.matmul(
                            bq_ps,
                            _r(ones65[bp:bp + 1, :]),
                            _r(rt[bp:bp + 1, s0:s0 + NT]),
                            start=True, stop=True,
                        )
                        use_gpsimd = l >= (3 if n % 2 == 0 else 2)
                        if use_gpsimd:
                            bq_sb = bqsb_pool.tile([P, NT], F32, tag="bq")
                            nc.scalar.copy(out=bq_sb, in_=bq_ps)
                        for kc in range(KC):
                            w_sb = wpool.tile([P, NT], MF, tag="w")
                            if use_gpsimd:
                                nc.gpsimd.tensor_mul(
                                    w_sb,
                                    _f(x_sb[:, kc, s0:s0 + NT]),
                                    bq_sb,
                                )
                            else:
                                nc.vector.tensor_mul(
                                    w_sb,
                                    _f(x_sb[:, kc, s0:s0 + NT]),
                                    bq_ps,
                                )
                            w_tiles.append(w_sb)

                    # scores: pscore[mc] = sum_{l,kc} kwT[l,kc,mc].T @ w[l,kc]
                    #         + kb[:,mc].T @ qssh[:, t0:t0+NT]
                    pscores = []
                    for mc in range(MC):
                        ps = pscore_pool.tile([P, NT], F32, tag="ps")
                        for l in range(L):
                            for kc in range(KC):
                                nc.tensor.matmul(
                                    ps,
                                    _r(kwT[:, l, kc, mc, :]),
                                    _r(w_tiles[l * KC + kc]),
                                    start=(l == 0 and kc == 0),
                                    stop=False,
                                )
                        nc.tensor.matmul(
                            ps,
                            _r(kb[:, mc, :]),
                            _r(qssh_sb[:, t0:t0 + NT]),
                            start=False, stop=True,
                        )
                        pscores.append(ps)

                    # softmax over channels, log-sum-exp form:
                    # w = exp(s - m - ln(sum(exp(s - m)))).
                    s_sbs = []
                    for mc in range(MC):
                        s_sb = mpool.tile([P, NT], F32, tag=f"s{mc}")
                        nc.scalar.copy(out=s_sb, in_=pscores[mc])
                        s_sbs.append(s_sb)
                    m1 = mpool.tile([P, NT], F32, tag="m1")
                    nc.vector.tensor_max(m1, s_sbs[0], s_sbs[1])
                    m1T = paux.tile([P, NT // P, P], F32, tag="paux")
                    for i in range(NT // P):
                        nc.tensor.transpose(
                            m1T[:, i, :], m1[:, i * P:(i + 1) * P], ident
                        )
                    maxT = mpool.tile([P, NT // P], F32, tag="maxT")
                    nc.vector.tensor_reduce(
                        out=maxT, in_=m1T, axis=mybir.AxisListType.X,
                        op=ALU.max,
                    )
                    mT = paux.tile([1, NT // P, P], F32, tag="paux")
                    for i in range(NT // P):
                        nc.tensor.transpose(mT[:, i, :], maxT[:, i:i + 1], ident)
                    mrow = mpool.tile([1, NT], MF, tag="mrow")
                    nc.scalar.copy(out=mrow, in_=mT)
                    mbc_ps = paux.tile([P, NT], F32, tag="paux")
                    nc.tensor.matmul(
                        mbc_ps, _r(ones65[0:1, :]), _r(mrow), start=True, stop=True
                    )

                    e_sbs = []
                    eins = []
                    for mc in range(MC):
                        ein = einpool.tile([P, NT], F32, tag="ein")
                        nc.vector.tensor_sub(ein, s_sbs[mc], mbc_ps)
                        e_sb = epool.tile([P, NT], MF, tag="e")
                        nc.scalar.activation(out=e_sb, in_=ein, func=AF.Exp)
                        e_sbs.append(e_sb)
                        eins.append(ein)

                    sum_ps = paux.tile([1, NT], F32, tag="paux")
                    for mc in range(MC):
                        nc.tensor.matmul(
                            sum_ps, _r(ones_col), _r(e_sbs[mc]),
                            start=(mc == 0), stop=(mc == MC - 1),
                        )
                    lnrow = rpool.tile([1, NT], MF, tag="lnrow")
                    nc.scalar.activation(out=lnrow, in_=sum_ps, func=AF.Ln)
                    lnbc_ps = paux.tile([P, NT], F32, tag="paux")
                    nc.tensor.matmul(
                        lnbc_ps, _r(ones65[0:1, :]), _r(lnrow), start=True,
                        stop=True,
                    )

                    # out = v * exp(ein - ln sum)
                    for mc in range(MC):
                        ein2 = wspool.tile([P, NT], F32, tag="wsoft")
                        nc.vector.tensor_sub(ein2, eins[mc], lnbc_ps)
                        w_soft = epool.tile([P, NT], F32, tag="ws2")
                        nc.scalar.activation(out=w_soft, in_=ein2, func=AF.Exp)
                        o_sb = opool.tile([P, NT], F32, tag="o")
                        if mc == 0:
                            nc.vector.tensor_mul(o_sb, w_soft, v_sbs[mc])
                        else:
                            nc.gpsimd.tensor_mul(o_sb, w_soft, v_sbs[mc])
                        nc.sync.dma_start(
                            out=y_out[b, mc, :, t0:t0 + NT], in_=o_sb
                        )
    nc.compile()
    return nc


def fold_weights(inputs: dict) -> dict:
    """Host-side folding of the tiny weight tensors into device layouts."""
    k_w = np.asarray(inputs["k_w"], np.float32)
    k_b = np.asarray(inputs["k_b"], np.float32)
    q_w = np.asarray(inputs["q_w"], np.float32)
    q_b = np.asarray(inputs["q_b"], np.float32)
    v_w = np.asarray(inputs["v_w"], np.float32)
    v_b = np.asarray(inputs["v_b"], np.float32)
    gamma = np.asarray(inputs["bn_gamma"], np.float32)
    beta = np.asarray(inputs["bn_beta"], np.float32)
    mean = np.asarray(inputs["bn_mean"], np.float32)
    var = np.asarray(inputs["bn_var"], np.float32)

    # kwT[p, l, kc, mc, m] = k_w[l, mc*128+m, kc*128+p]
    kwT = np.ascontiguousarray(
        k_w.reshape(L, MC, P, KC, P).transpose(4, 0, 3, 1, 2)
    )
    kb = np.ascontiguousarray(k_b.reshape(L, MC, P))
    qws = q_w.sum(axis=1)                       # [L, C]
    qwsT = np.ascontiguousarray(qws.reshape(L, KC, P).transpose(2, 1, 0))
    qbs = np.ascontiguousarray(q_b.sum(axis=1).reshape(L, 1))
    scale = gamma / np.sqrt(var + BN_EPS)
    vw_f = v_w * scale[:, None]
    vb_f = (v_b - mean) * scale + beta
    if PRELU_EXPLICIT:
        alpha = float(np.asarray(inputs["prelu_alpha"]).reshape(-1)[0])
        a = (1.0 + alpha) / 2.0
        vw_f = vw_f * a
        vb_f = vb_f * a
    vwT = np.ascontiguousarray(
        vw_f.reshape(MC, P, KC, P).transpose(3, 2, 0, 1)
    )
    vbT = np.ascontiguousarray(vb_f.reshape(MC, P).transpose(1, 0))
    return {
        "kwT": kwT, "kb": kb, "qwsT": qwsT, "qbs": qbs,
        "vwT": vwT, "vb": vbT, "ident": np.eye(P, dtype=np.float32),
        "ones65": np.ones((65, P), np.float32),
        "ones_col": np.ones((P, 1), np.float32),
        "zeros8": np.zeros((P, KC, 8), np.float32),
    }


_CACHE: dict = {}


def kernel(**inputs) -> np.ndarray:
    x = np.ascontiguousarray(np.asarray(inputs["x"], np.float32))
    alpha = float(np.asarray(inputs["prelu_alpha"]).reshape(-1)[0])

    key = ("prog", alpha, PRELU_EXPLICIT)
    if key not in _CACHE:
        _CACHE[key] = build_program(alpha)
    nc = _CACHE[key]

    weights = fold_weights(inputs)
    core_ids = list(range(NCORES))
    in_maps = []
    for i in range(NCORES):
        xs = x[i * BPC:(i + 1) * BPC].reshape(BPC, KC, P, T)
        in_maps.append({"x": np.ascontiguousarray(xs), **weights})

    res = run_bass_kernel_spmd(nc, in_maps, core_ids)
    outs = [r["y"].reshape(BPC, C, T) for r in res.results]
    return np.concatenate(outs, axis=0)


if __name__ == "__main__":
    rng = np.random.default_rng(0)
    demo = {
        "x": rng.standard_normal((B, C, T), dtype=np.float32),
        "q_w": rng.standard_normal((L, C, C), dtype=np.float32) / 16,
        "q_b": rng.standard_normal((L, C), dtype=np.float32) * 0.02,
        "k_w": rng.standard_normal((L, C, C), dtype=np.float32) / 16,
        "k_b": rng.standard_normal((L, C), dtype=np.float32) * 0.02,
        "v_w": rng.standard_normal((C, C), dtype=np.float32) / 16,
        "v_b": rng.standard_normal((C,), dtype=np.float32) * 0.02,
        "bn_gamma": rng.uniform(0.5, 1.5, C).astype(np.float32),
        "bn_beta": rng.standard_normal(C).astype(np.float32) * 0.02,
        "bn_mean": rng.standard_normal(C).astype(np.float32) * 0.1,
        "bn_var": rng.uniform(0.5, 1.5, C).astype(np.float32),
        "prelu_alpha": np.full((1,), 0.25, np.float32),
    }
    y = kernel(**demo)
    print("out", y.shape, y.dtype, float(np.abs(y).max()))



# revision 7
# speedup vs baseline: 1.8004x; 1.8004x over previous
"""Trainium2 Bass kernel for nn_ChannelAttention (B=16, C=256, T=2048, L=5).

Data-parallel over 8 NeuronCores: each core processes 2 batches.

Math (per batch b):
  qsum[l,t]   = qws[l] @ x[:,t] + qbs[l]                      (qws = q_w.sum(1))
  scores[c,t] = sum_l (k_w[l] @ (x * Bqsum[l]))[c, t-l] + sum_l k_b[l,c]*qsum[l,t-l]
  w = softmax_c(scores);  v = PReLU(BN(v_w @ x + v_b));  out = w * v

v2 design (fp16 datapath, validated 6.4e-3 rel err vs 2e-2 budget):
  - everything that feeds the PE is fp16 (1 cyc/row matmuls, FWL weight loads,
    half the DMA/SBUF traffic); PSUM accumulation stays fp32.
  - bq (qsum row broadcast over 128 partitions) via K=1 matmuls ROW-PACKED with
    tile_position at base partitions 0/32/64 (3+2 concurrent waves).
  - gating multiplies on DVE in fp16 (2x mode, ~330ns/tile); an x copy shifted
    by one element (x16o) keeps odd-lag windows 4B-aligned for 2x mode.
  - softmax over channels in the TRANSPOSED domain: PE-transpose scores,
    DVE free-dim max-reduce, ACT Exp with per-partition bias=-max and fused
    accum_out channel sums, tiny DVE reciprocal [128,4], per-partition
    tensor_scalar renormalize, PE-transpose back.  No ones-matmul broadcasts,
    no Ln (kills ACT table thrash), no [1,T]-row ops.
  - v phase grouped per batch so ACT Prelu<->Exp table switches happen at most
    twice per batch.
"""

import sys

sys.path.insert(0, "/opt/trn_rl_repo")

import numpy as np

import concourse.bass as bass
import concourse.mybir as mybir
import concourse.tile as tile
from concourse import bacc
from concourse.bass_utils import run_bass_kernel_spmd

B, C, T, L = 16, 256, 2048, 5
NCORES = 8
BPC = B // NCORES      # batches per core
P = 128                # partitions
KC = C // P            # k chunks (2)
MC = C // P            # m chunks (2)
NT = 512               # time tile
NB = NT // P           # transpose blocks per time tile (4)
NCHUNK = T // NT       # 4
PAD = 8                # left zero pad (t<0 lag windows)
TP = PAD + T           # padded time length
QPITCH = T + 16        # dram scratch row pitch for shifted qsum
BN_EPS = 1e-5

F32 = mybir.dt.float32
F32R = mybir.dt.float32r
F16 = mybir.dt.float16

# CoreSim lacks the Prelu activation: the sim path computes
# v = max(z, alpha*z) with two DVE ops instead.
PRELU_EXPLICIT = False

AF = mybir.ActivationFunctionType
ALU = mybir.AluOpType


def build_program(alpha: float) -> bass.Bass:
    nc = bacc.Bacc("TRN2", target_bir_lowering=False, debug=False, num_devices=NCORES)

    x_in = nc.dram_tensor("x", [BPC, KC, P, TP], F16, kind="ExternalInput").ap()
    kwT_in = nc.dram_tensor("kwT", [P, L, KC, MC, P], F16, kind="ExternalInput").ap()
    kb_in = nc.dram_tensor("kb", [L, MC, P], F16, kind="ExternalInput").ap()
    qwsT_in = nc.dram_tensor("qwsT", [P, KC, L], F16, kind="ExternalInput").ap()
    qbs_in = nc.dram_tensor("qbs", [L, 1], F32, kind="ExternalInput").ap()
    vwT_in = nc.dram_tensor("vwT", [P, KC, MC, P], F16, kind="ExternalInput").ap()
    vb_in = nc.dram_tensor("vb", [P, MC], F32, kind="ExternalInput").ap()
    ident_in = nc.dram_tensor("ident", [P, P], F32R, kind="ExternalInput").ap()
    ident16_in = nc.dram_tensor("ident16", [P, P], F16, kind="ExternalInput").ap()
    ones97_in = nc.dram_tensor("ones97", [97, P], F16, kind="ExternalInput").ap()
    y_out = nc.dram_tensor("y", [BPC, MC, P, T], F32, kind="ExternalOutput").ap()
    # scratch for the lag-shift of qsum rows (row l shifted right by l)
    qsd = nc.dram_tensor("qs_scratch", [BPC, L, QPITCH], F16).ap()

    from contextlib import ExitStack

    with tile.TileContext(nc) as tc:
        with ExitStack() as ctx:
            ep = ctx.enter_context
            ep(nc.allow_low_precision(
                reason="fp16 datapath validated at 6.4e-3 rel err vs the "
                       "2e-2 budget; PSUM accumulation stays fp32"
            ))
            consts = ep(tc.tile_pool(name="consts", bufs=1))
            xpool = ep(tc.tile_pool(name="xpool", bufs=2))
            qspool = ep(tc.tile_pool(name="qspool", bufs=2))
            qsshpool = ep(tc.tile_pool(name="qsshpool", bufs=2))
            qrowpool = ep(tc.tile_pool(name="qrowpool", bufs=2))
            vpool = ep(tc.tile_pool(name="vpool", bufs=12))
            bqpool = ep(tc.tile_pool(name="bqpool", bufs=8))
            wpool = ep(tc.tile_pool(name="wpool", bufs=12))
            spool = ep(tc.tile_pool(name="spool", bufs=4))
            epool = ep(tc.tile_pool(name="epool", bufs=6))
            accpool = ep(tc.tile_pool(name="accpool", bufs=12))
            opool = ep(tc.tile_pool(name="opool", bufs=4))
            # PSUM: 8 banks.  pscore 2 ([128,512] each), pbq 3, pT 3.
            pscore_pool = ep(tc.tile_pool(name="pscore", bufs=2, space="PSUM"))
            pbq_pool = ep(tc.tile_pool(name="pbq", bufs=3, space="PSUM"))
            pT_pool = ep(tc.tile_pool(name="pT", bufs=3, space="PSUM"))

            # ---- constants ----
            kwT = consts.tile([P, L, KC, MC, P], F16)
            nc.sync.dma_start(out=kwT, in_=kwT_in)
            kb = consts.tile([L, MC, P], F16)
            nc.sync.dma_start(out=kb, in_=kb_in)
            qwsT = consts.tile([P, KC, L], F16)
            nc.sync.dma_start(out=qwsT, in_=qwsT_in)
            qbs = consts.tile([L, 1], F32)
            nc.sync.dma_start(out=qbs, in_=qbs_in)
            vwT = consts.tile([P, KC, MC, P], F16)
            nc.sync.dma_start(out=vwT, in_=vwT_in)
            vb = consts.tile([P, MC], F32)
            nc.sync.dma_start(out=vb, in_=vb_in)
            ones97 = consts.tile([97, P], F16)       # lhsT rows for bcast mm
            nc.sync.dma_start(out=ones97, in_=ones97_in)
            ident = consts.tile([P, P], F32R)        # PE transpose (fp32r scores)
            nc.sync.dma_start(out=ident, in_=ident_in)
            ident16 = consts.tile([P, P], F16)       # PE transpose (fp16 weights)
            nc.sync.dma_start(out=ident16, in_=ident16_in)
            zpad = consts.tile([L, PAD], F16)        # zero left pad for qsd
            nc.vector.memset(zpad, 0.0)


            for b in range(BPC):
                # ---- load x (and the 1-shifted copy for odd-lag alignment) ----
                x_sb = xpool.tile([P, KC, TP], F16, tag="x")
                xo_sb = xpool.tile([P, KC, TP], F16, tag="xo")
                for kc in range(KC):
                    h = TP // 2
                    nc.sync.dma_start(out=x_sb[:, kc, 0:h], in_=x_in[b, kc, :, 0:h])
                    nc.sync.dma_start(out=x_sb[:, kc, h:TP], in_=x_in[b, kc, :, h:TP])
                    # xo[j] = x[j+1]  (even element offsets for odd lags)
                    nc.sync.dma_start(
                        out=xo_sb[:, kc, 0:TP - 1], in_=x_in[b, kc, :, 1:TP]
                    )

                # ---- qsum rows: qs[l,t] = qws[l] @ x[:,t] + qbs[l] ----
                qs_sb = qspool.tile([L, T], F16, tag="qs")
                for n in range(NCHUNK):
                    qs_ps = pbq_pool.tile([L, NT], F32, tag="pbq")
                    for kc in range(KC):
                        nc.tensor.matmul(
                            qs_ps,
                            qwsT[:, kc, :],
                            x_sb[:, kc, PAD + n * NT:PAD + (n + 1) * NT],
                            start=(kc == 0),
                            stop=(kc == KC - 1),
                        )
                    nc.vector.tensor_scalar_add(
                        qs_sb[:, n * NT:(n + 1) * NT], qs_ps, qbs
                    )

                # ---- shifted qsum via DRAM round trip ----
                nc.sync.dma_start(out=qsd[b, :, PAD:PAD + T], in_=qs_sb)
                nc.sync.dma_start(out=qsd[b, :, 0:PAD], in_=zpad)
                qssh_sb = qsshpool.tile([L, T], F16, tag="qssh")
                shifted = bass.AP(
                    tensor=qsd.tensor,
                    offset=b * L * QPITCH + PAD,
                    ap=[[QPITCH - 1, L], [1, T]],
                )
                nc.sync.dma_start(out=qssh_sb, in_=shifted)
                # unshifted qsum rows at base partitions 0/32/64 (+0/32) for
                # the row-packed K=1 broadcast matmuls
                qrowA = qrowpool.tile([65, TP], F16, tag="qrA")
                qrowB = qrowpool.tile([33, TP], F16, tag="qrB")
                for l in range(L):
                    rt, bp = (qrowA, 32 * l) if l < 3 else (qrowB, 32 * (l - 3))
                    nc.sync.dma_start(
                        out=rt[bp:bp + 1, :], in_=qsd[b, l:l + 1, 0:TP]
                    )

                # ---- v phase (grouped: one Prelu table window per batch) ----
                v_sbs = {}
                for n in range(NCHUNK):
                    t0 = n * NT
                    for mc in range(MC):
                        v_ps = pT_pool.tile([P, NT], F32, tag="pT")
                        for kc in range(KC):
                            nc.tensor.matmul(
                                v_ps,
                                vwT[:, kc, mc, :],
                                x_sb[:, kc, PAD + t0:PAD + t0 + NT],
                                start=(kc == 0),
                                stop=(kc == KC - 1),
                            )
                        v_sb = vpool.tile([P, NT], F16, tag="v")
                        if PRELU_EXPLICIT:
                            vz = spool.tile([P, NT], F32, tag="vz")
                            nc.vector.tensor_scalar_add(vz, v_ps, vb[:, mc:mc + 1])
                            nc.vector.scalar_tensor_tensor(
                                v_sb, vz, float(alpha), vz, ALU.mult, ALU.max
                            )
                        else:
                            nc.scalar.activation(
                                out=v_sb, in_=v_ps, func=AF.Prelu,
                                bias=vb[:, mc:mc + 1], scale=1.0, alpha=alpha,
                            )
                        v_sbs[(n, mc)] = v_sb

                # ---- time-chunk loop ----
                for n in range(NCHUNK):
                    t0 = n * NT
                    s0 = PAD + t0

                    # bq: row-packed K=1 broadcast matmuls (waves of 3 + 2)
                    bq16 = []
                    for l in range(L):
                        rt, bp = (qrowA, 32 * l) if l < 3 else (qrowB, 32 * (l - 3))
                        bq_ps = pbq_pool.tile([P, NT], F32, tag="pbq")
                        nc.tensor.matmul(
                            bq_ps,
                            ones97[bp:bp + 1, :],
                            rt[bp:bp + 1, s0 - l:s0 - l + NT],
                            start=True, stop=True,
                        )
                        bq_sb = bqpool.tile([P, NT], F16, tag="bq")
                        nc.scalar.copy(out=bq_sb, in_=bq_ps)
                        bq16.append(bq_sb)

                    # gated tiles: w_l[:, i] = x[:, s0-l+i] * qsum[l, s0-l+i]
                    w_tiles = []
                    for l in range(L):
                        for kc in range(KC):
                            w_sb = wpool.tile([P, NT], F16, tag="w")
                            if l % 2 == 0:
                                xa = x_sb[:, kc, s0 - l:s0 - l + NT]
                            else:
                                xa = xo_sb[:, kc, s0 - l - 1:s0 - l - 1 + NT]
                            if l >= 3:
                                nc.gpsimd.tensor_mul(w_sb, xa, bq16[l])
                            else:
                                nc.vector.tensor_mul(w_sb, xa, bq16[l])
                            w_tiles.append(w_sb)

                    # scores: ps[mc] = sum_{l,kc} kwT[l,kc,mc].T @ w[l,kc]
                    #         + kb[:,mc].T @ qssh[:, t0:t0+NT]
                    pscores = []
                    for mc in range(MC):
                        ps = pscore_pool.tile([P, NT], F32, tag="ps")
                        for l in range(L):
                            for kc in range(KC):
                                nc.tensor.matmul(
                                    ps,
                                    kwT[:, l, kc, mc, :],
                                    w_tiles[l * KC + kc],
                                    start=(l == 0 and kc == 0),
                                    stop=False,
                                )
                        nc.tensor.matmul(
                            ps,
                            kb[:, mc, :],
                            qssh_sb[:, t0:t0 + NT],
                            start=False, stop=True,
                        )
                        pscores.append(ps)

                    # ---- softmax over channels in the transposed domain ----
                    s_sbs = []
                    sTs = []
                    for mc in range(MC):
                        s_sb = spool.tile([P, NT], F32R, tag="s")
                        nc.scalar.copy(out=s_sb, in_=pscores[mc])
                        s_sbs.append(s_sb)
                        sT = pT_pool.tile([P, NB, P], F32R, tag="pT")
                        for i in range(NB):
                            nc.tensor.transpose(
                                sT[:, i, :], s_sb[:, i * P:(i + 1) * P], ident
                            )
                        sTs.append(sT.bitcast(F32))
                    maxTs = []
                    for mc in range(MC):
                        maxT = accpool.tile([P, NB], F32, tag="maxT")
                        nc.vector.tensor_reduce(
                            out=maxT, in_=sTs[mc], axis=mybir.AxisListType.X,
                            op=ALU.max,
                        )
                        maxTs.append(maxT)
                    # nmax = -max(maxT0, maxT1) = min(-maxT0, -maxT1)
                    nm1 = accpool.tile([P, NB], F32, tag="nm1")
                    nc.vector.tensor_scalar_mul(nm1, maxTs[1], -1.0)
                    nmax = accpool.tile([P, NB], F32, tag="nmax")
                    nc.vector.scalar_tensor_tensor(
                        nmax, maxTs[0], -1.0, nm1, ALU.mult, ALU.min
                    )
                    # e = exp(sT - max), fused per-block channel sums
                    eTs = []
                    accs = []
                    for mc in range(MC):
                        eT = epool.tile([P, NB, P], F16, tag="eT")
                        acc = accpool.tile([P, NB], F32, tag="acc")
                        for i in range(NB):
                            nc.scalar.activation(
                                out=eT[:, i, :], in_=sTs[mc][:, i, :],
                                func=AF.Exp, bias=nmax[:, i:i + 1], scale=1.0,
                                accum_out=acc[:, i:i + 1],
                            )
                        eTs.append(eT)
                        accs.append(acc)
                    sums = accpool.tile([P, NB], F32, tag="sums")
                    nc.vector.tensor_add(sums, accs[0], accs[1])
                    rT = accpool.tile([P, NB], F32, tag="rT")
                    nc.vector.reciprocal(rT, sums)
                    # renormalize + transpose back to channel-major
                    for mc in range(MC):
                        wT = epool.tile([P, NB, P], F16, tag="wT")
                        for i in range(NB):
                            nc.vector.tensor_scalar_mul(
                                wT[:, i, :], eTs[mc][:, i, :], rT[:, i:i + 1]
                            )
                        wb_ps = pT_pool.tile([P, NB, P], F16, tag="pT")
                        for i in range(NB):
                            nc.tensor.transpose(
                                wb_ps[:, i, :], wT[:, i, :], ident16
                            )
                        o_sb = opool.tile([P, NT], F32, tag="o")
                        nc.vector.tensor_mul(o_sb, wb_ps, v_sbs[(n, mc)])
                        nc.sync.dma_start(
                            out=y_out[b, mc, :, t0:t0 + NT], in_=o_sb
                        )
    nc.compile()
    return nc


def fold_weights(inputs: dict) -> dict:
    """Host-side folding of the tiny weight tensors into device layouts."""
    k_w = np.asarray(inputs["k_w"], np.float32)
    k_b = np.asarray(inputs["k_b"], np.float32)
    q_w = np.asarray(inputs["q_w"], np.float32)
    q_b = np.asarray(inputs["q_b"], np.float32)
    v_w = np.asarray(inputs["v_w"], np.float32)
    v_b = np.asarray(inputs["v_b"], np.float32)
    gamma = np.asarray(inputs["bn_gamma"], np.float32)
    beta = np.asarray(inputs["bn_beta"], np.float32)
    mean = np.asarray(inputs["bn_mean"], np.float32)
    var = np.asarray(inputs["bn_var"], np.float32)

    # kwT[p, l, kc, mc, m] = k_w[l, mc*128+m, kc*128+p]
    kwT = np.ascontiguousarray(
        k_w.reshape(L, MC, P, KC, P).transpose(4, 0, 3, 1, 2)
    ).astype(np.float16)
    kb = np.ascontiguousarray(k_b.reshape(L, MC, P)).astype(np.float16)
    qws = q_w.sum(axis=1)                       # [L, C]
    qwsT = np.ascontiguousarray(
        qws.reshape(L, KC, P).transpose(2, 1, 0)
    ).astype(np.float16)
    qbs = np.ascontiguousarray(q_b.sum(axis=1).reshape(L, 1))
    scale = gamma / np.sqrt(var + BN_EPS)
    vw_f = v_w * scale[:, None]
    vb_f = (v_b - mean) * scale + beta
    vwT = np.ascontiguousarray(
        vw_f.reshape(MC, P, KC, P).transpose(3, 2, 0, 1)
    ).astype(np.float16)
    vbT = np.ascontiguousarray(vb_f.reshape(MC, P).transpose(1, 0))
    return {
        "kwT": kwT, "kb": kb, "qwsT": qwsT, "qbs": qbs,
        "vwT": vwT, "vb": vbT,
        "ident": np.eye(P, dtype=np.float32),
        "ident16": np.eye(P, dtype=np.float16),
        "ones97": np.ones((97, P), np.float16),
    }


_CACHE: dict = {}


def make_in_maps(inputs: dict) -> list:
    weights = fold_weights(inputs)
    x = np.asarray(inputs["x"], np.float32)
    # pad x on the left with zeros and convert to fp16
    xp = np.zeros((B, C, TP), np.float16)
    xp[:, :, PAD:] = x.astype(np.float16)
    xp = xp.reshape(B // BPC, BPC, KC, P, TP)
    return [
        {"x": np.ascontiguousarray(xp[i]), **weights} for i in range(NCORES)
    ]


def kernel(**inputs) -> np.ndarray:
    alpha = float(np.asarray(inputs["prelu_alpha"]).reshape(-1)[0])

    key = ("prog", alpha, PRELU_EXPLICIT)
    if key not in _CACHE:
        _CACHE[key] = build_program(alpha)
    nc = _CACHE[key]

    in_maps = make_in_maps(inputs)
    res = run_bass_kernel_spmd(nc, in_maps, list(range(NCORES)))
    outs = [r["y"].reshape(BPC, C, T) for r in res.results]
    return np.concatenate(outs, axis=0)


if __name__ == "__main__":
    rng = np.random.default_rng(0)
    demo = {
        "x": rng.standard_normal((B, C, T), dtype=np.float32),
        "q_w": rng.standard_normal((L, C, C), dtype=np.float32) / 16,
        "q_b": rng.standard_normal((L, C), dtype=np.float32) * 0.02,
        "k_w": rng.standard_normal((L, C, C), dtype=np.float32) / 16,
        "k_b": rng.standard_normal((L, C), dtype=np.float32) * 0.02,
        "v_w": rng.standard_normal((C, C), dtype=np.float32) / 16,
        "v_b": rng.standard_normal((C,), dtype=np.float32) * 0.02,
        "bn_gamma": rng.uniform(0.5, 1.5, C).astype(np.float32),
        "bn_beta": rng.standard_normal(C).astype(np.float32) * 0.02,
        "bn_mean": rng.standard_normal(C).astype(np.float32) * 0.1,
        "bn_var": rng.uniform(0.5, 1.5, C).astype(np.float32),
        "prelu_alpha": np.full((1,), 0.25, np.float32),
    }
    y = kernel(**demo)
    print("out", y.shape, y.dtype, float(np.abs(y).max()))


# revision 9
# speedup vs baseline: 1.9174x; 1.0650x over previous
"""Trainium2 Bass kernel for nn_ChannelAttention (B=16, C=256, T=2048, L=5).

Data-parallel over 8 NeuronCores: each core processes 2 batches.

Math (per batch b):
  qsum[l,t]   = qws[l] @ x[:,t] + qbs[l]                      (qws = q_w.sum(1))
  scores[c,t] = sum_l (k_w[l] @ (x * Bqsum[l]))[c, t-l] + sum_l k_b[l,c]*qsum[l,t-l]
  w = softmax_c(scores);  v = PReLU(BN(v_w @ x + v_b));  out = w * v

v2 design (fp16 datapath, validated 6.4e-3 rel err vs 2e-2 budget):
  - everything that feeds the PE is fp16 (1 cyc/row matmuls, FWL weight loads,
    half the DMA/SBUF traffic); PSUM accumulation stays fp32.
  - bq (qsum row broadcast over 128 partitions) via K=1 matmuls ROW-PACKED with
    tile_position at base partitions 0/32/64 (3+2 concurrent waves).
  - gating multiplies on DVE in fp16 (2x mode, ~330ns/tile); an x copy shifted
    by one element (x16o) keeps odd-lag windows 4B-aligned for 2x mode.
  - softmax over channels in the TRANSPOSED domain: PE-transpose scores,
    DVE free-dim max-reduce, ACT Exp with per-partition bias=-max and fused
    accum_out channel sums, tiny DVE reciprocal [128,4], per-partition
    tensor_scalar renormalize, PE-transpose back.  No ones-matmul broadcasts,
    no Ln (kills ACT table thrash), no [1,T]-row ops.
  - v phase grouped per batch so ACT Prelu<->Exp table switches happen at most
    twice per batch.
"""

import sys

sys.path.insert(0, "/opt/trn_rl_repo")

import numpy as np

import concourse.bass as bass
import concourse.mybir as mybir
import concourse.tile as tile
from concourse import bacc
from concourse.bass_utils import run_bass_kernel_spmd

B, C, T, L = 16, 256, 2048, 5
NCORES = 8
BPC = B // NCORES      # batches per core
P = 128                # partitions
KC = C // P            # k chunks (2)
MC = C // P            # m chunks (2)
NT = 512               # time tile
NB = NT // P           # transpose blocks per time tile (4)
NCHUNK = T // NT       # 4
PAD = 8                # left zero pad (t<0 lag windows)
TP = PAD + T           # padded time length
QPITCH = T + 16        # dram scratch row pitch for shifted qsum
BN_EPS = 1e-5

F32 = mybir.dt.float32
F32R = mybir.dt.float32r
F16 = mybir.dt.float16

# CoreSim lacks the Prelu activation: the sim path computes
# v = max(z, alpha*z) with two DVE ops instead.
PRELU_EXPLICIT = False

AF = mybir.ActivationFunctionType
ALU = mybir.AluOpType


def build_program(alpha: float) -> bass.Bass:
    nc = bacc.Bacc("TRN2", target_bir_lowering=False, debug=False, num_devices=NCORES)

    x_in = nc.dram_tensor("x", [BPC, KC, P, TP], F16, kind="ExternalInput").ap()
    kwT_in = nc.dram_tensor("kwT", [P, L, KC, MC, P], F16, kind="ExternalInput").ap()
    kb_in = nc.dram_tensor("kb", [L, MC, P], F16, kind="ExternalInput").ap()
    qwsT_in = nc.dram_tensor("qwsT", [P, KC, L], F16, kind="ExternalInput").ap()
    qbs_in = nc.dram_tensor("qbs", [L, 1], F32, kind="ExternalInput").ap()
    vwT_in = nc.dram_tensor("vwT", [P, KC, MC, P], F16, kind="ExternalInput").ap()
    vb_in = nc.dram_tensor("vb", [P, MC], F32, kind="ExternalInput").ap()
    ident_in = nc.dram_tensor("ident", [P, P], F32R, kind="ExternalInput").ap()
    ident16_in = nc.dram_tensor("ident16", [P, P], F16, kind="ExternalInput").ap()
    y_out = nc.dram_tensor("y", [BPC, MC, P, T], F16, kind="ExternalOutput").ap()
    # scratch for the lag-shift of qsum rows (row l shifted right by l)
    qsd = nc.dram_tensor("qs_scratch", [BPC, L, QPITCH], F16).ap()

    from contextlib import ExitStack

    with tile.TileContext(nc) as tc:
        with ExitStack() as ctx:
            ep = ctx.enter_context
            ep(nc.allow_low_precision(
                reason="fp16 datapath validated at 6.4e-3 rel err vs the "
                       "2e-2 budget; PSUM accumulation stays fp32"
            ))
            consts = ep(tc.tile_pool(name="consts", bufs=1))
            xpool = ep(tc.tile_pool(name="xpool", bufs=2))
            qspool = ep(tc.tile_pool(name="qspool", bufs=2))
            qsshpool = ep(tc.tile_pool(name="qsshpool", bufs=2))
            bqlpool = ep(tc.tile_pool(name="bqlpool", bufs=10))
            vpool = ep(tc.tile_pool(name="vpool", bufs=12))
            wpool = ep(tc.tile_pool(name="wpool", bufs=12))
            spool = ep(tc.tile_pool(name="spool", bufs=4))
            epool = ep(tc.tile_pool(name="epool", bufs=6))
            accpool = ep(tc.tile_pool(name="accpool", bufs=12))
            opool = ep(tc.tile_pool(name="opool", bufs=4))
            # PSUM: 8 banks.  pscore 3 ([128,512] each), pbq 2, pT 3.
            pscore_pool = ep(tc.tile_pool(name="pscore", bufs=3, space="PSUM"))
            pbq_pool = ep(tc.tile_pool(name="pbq", bufs=2, space="PSUM"))
            pT_pool = ep(tc.tile_pool(name="pT", bufs=3, space="PSUM"))

            # ---- constants ----
            kwT = consts.tile([P, L, KC, MC, P], F16)
            nc.sync.dma_start(out=kwT, in_=kwT_in)
            kb = consts.tile([L, MC, P], F16)
            nc.sync.dma_start(out=kb, in_=kb_in)
            qwsT = consts.tile([P, KC, L], F16)
            nc.sync.dma_start(out=qwsT, in_=qwsT_in)
            qbs = consts.tile([L, 1], F32)
            nc.sync.dma_start(out=qbs, in_=qbs_in)
            vwT = consts.tile([P, KC, MC, P], F16)
            nc.sync.dma_start(out=vwT, in_=vwT_in)
            vb = consts.tile([P, MC], F32)
            nc.sync.dma_start(out=vb, in_=vb_in)
            ident = consts.tile([P, P], F32R)        # PE transpose (fp32r scores)
            nc.sync.dma_start(out=ident, in_=ident_in)
            ident16 = consts.tile([P, P], F16)       # PE transpose (fp16 weights)
            nc.sync.dma_start(out=ident16, in_=ident16_in)
            zpad = consts.tile([L, PAD], F16)        # zero left pad for qsd
            nc.vector.memset(zpad, 0.0)


            for b in range(BPC):
                # ---- load x (and the 1-shifted copy for odd-lag alignment) ----
                x_sb = xpool.tile([P, KC, TP], F16, tag="x")
                xo_sb = xpool.tile([P, KC, TP], F16, tag="xo")
                for kc in range(KC):
                    h = TP // 2
                    nc.sync.dma_start(out=x_sb[:, kc, 0:h], in_=x_in[b, kc, :, 0:h])
                    nc.sync.dma_start(out=x_sb[:, kc, h:TP], in_=x_in[b, kc, :, h:TP])
                    # xo[j] = x[j+1]  (even element offsets for odd lags)
                    nc.sync.dma_start(
                        out=xo_sb[:, kc, 0:TP - 1], in_=x_in[b, kc, :, 1:TP]
                    )

                # ---- qsum rows: qs[l,t] = qws[l] @ x[:,t] + qbs[l] ----
                qs_sb = qspool.tile([L, T], F16, tag="qs")
                for n in range(NCHUNK):
                    qs_ps = pbq_pool.tile([L, NT], F32, tag="pbq")
                    for kc in range(KC):
                        nc.tensor.matmul(
                            qs_ps,
                            qwsT[:, kc, :],
                            x_sb[:, kc, PAD + n * NT:PAD + (n + 1) * NT],
                            start=(kc == 0),
                            stop=(kc == KC - 1),
                        )
                    nc.vector.tensor_scalar_add(
                        qs_sb[:, n * NT:(n + 1) * NT], qs_ps, qbs
                    )

                # ---- shifted qsum via DRAM round trip ----
                nc.sync.dma_start(out=qsd[b, :, PAD:PAD + T], in_=qs_sb)
                nc.sync.dma_start(out=qsd[b, :, 0:PAD], in_=zpad)
                nc.sync.dma_start(out=qsd[b, :, PAD + T:QPITCH], in_=zpad)
                qssh_sb = qsshpool.tile([L, T], F16, tag="qssh")
                shifted = bass.AP(
                    tensor=qsd.tensor,
                    offset=b * L * QPITCH + PAD,
                    ap=[[QPITCH - 1, L], [1, T]],
                )
                nc.sync.dma_start(out=qssh_sb, in_=shifted)
                # bql[l][p, j] = qsum[l, j-8-l]: the lag-shifted qsum row
                # broadcast to all 128 partitions via a stride-0-partition DMA.
                # (cols j<8 read the previous row's tail; never used.)
                bqls = []
                for l in range(L):
                    bql = bqlpool.tile([P, TP], F16, tag="bql")
                    bcast = bass.AP(
                        tensor=qsd.tensor,
                        offset=(b * L + l) * QPITCH - l,
                        ap=[[0, P], [1, TP]],
                    )
                    nc.sync.dma_start(out=bql, in_=bcast)
                    bqls.append(bql)

                # ---- v phase (grouped: one Prelu table window per batch) ----
                v_sbs = {}
                for n in range(NCHUNK):
                    t0 = n * NT
                    for mc in range(MC):
                        v_ps = pT_pool.tile([P, NT], F32, tag="pT")
                        for kc in range(KC):
                            nc.tensor.matmul(
                                v_ps,
                                vwT[:, kc, mc, :],
                                x_sb[:, kc, PAD + t0:PAD + t0 + NT],
                                start=(kc == 0),
                                stop=(kc == KC - 1),
                            )
                        v_sb = vpool.tile([P, NT], F16, tag="v")
                        if PRELU_EXPLICIT:
                            vz = spool.tile([P, NT], F32, tag="vz")
                            nc.vector.tensor_scalar_add(vz, v_ps, vb[:, mc:mc + 1])
                            nc.vector.scalar_tensor_tensor(
                                v_sb, vz, float(alpha), vz, ALU.mult, ALU.max
                            )
                        else:
                            nc.scalar.activation(
                                out=v_sb, in_=v_ps, func=AF.Prelu,
                                bias=vb[:, mc:mc + 1], scale=1.0, alpha=alpha,
                            )
                        v_sbs[(n, mc)] = v_sb

                # ---- time-chunk loop ----
                for n in range(NCHUNK):
                    t0 = n * NT
                    s0 = PAD + t0

                    # gated tiles: w_l[:, i] = x[:, s0-l+i] * qsum[l, t0-l+i]
                    w_tiles = []
                    for l in range(L):
                        for kc in range(KC):
                            w_sb = wpool.tile([P, NT], F16, tag="w")
                            if l % 2 == 0:
                                xa = x_sb[:, kc, s0 - l:s0 - l + NT]
                            else:
                                xa = xo_sb[:, kc, s0 - l - 1:s0 - l - 1 + NT]
                            bq = bqls[l][:, s0:s0 + NT]
                            if l >= 3:
                                nc.gpsimd.tensor_mul(w_sb, xa, bq)
                            else:
                                nc.vector.tensor_mul(w_sb, xa, bq)
                            w_tiles.append(w_sb)

                    # scores: ps[mc] = sum_{l,kc} kwT[l,kc,mc].T @ w[l,kc]
                    #         + kb[:,mc].T @ qssh[:, t0:t0+NT]
                    pscores = []
                    for mc in range(MC):
                        ps = pscore_pool.tile([P, NT], F32, tag="ps")
                        for l in range(L):
                            for kc in range(KC):
                                nc.tensor.matmul(
                                    ps,
                                    kwT[:, l, kc, mc, :],
                                    w_tiles[l * KC + kc],
                                    start=(l == 0 and kc == 0),
                                    stop=False,
                                )
                        nc.tensor.matmul(
                            ps,
                            kb[:, mc, :],
                            qssh_sb[:, t0:t0 + NT],
                            start=False, stop=True,
                        )
                        pscores.append(ps)

                    # ---- softmax over channels in the transposed domain ----
                    s_sbs = []
                    sTs = []
                    for mc in range(MC):
                        s_sb = spool.tile([P, NT], F32R, tag="s")
                        nc.scalar.copy(out=s_sb, in_=pscores[mc])
                        s_sbs.append(s_sb)
                        sT = pT_pool.tile([P, NB, P], F32R, tag="pT")
                        for i in range(NB):
                            nc.tensor.transpose(
                                sT[:, i, :], s_sb[:, i * P:(i + 1) * P], ident
                            )
                        sTs.append(sT.bitcast(F32))
                    maxTs = []
                    for mc in range(MC):
                        maxT = accpool.tile([P, NB], F32, tag="maxT")
                        nc.vector.tensor_reduce(
                            out=maxT, in_=sTs[mc], axis=mybir.AxisListType.X,
                            op=ALU.max,
                        )
                        maxTs.append(maxT)
                    # nmax = -max(maxT0, maxT1) = min(-maxT0, -maxT1)
                    nm1 = accpool.tile([P, NB], F32, tag="nm1")
                    nc.vector.tensor_scalar_mul(nm1, maxTs[1], -1.0)
                    nmax = accpool.tile([P, NB], F32, tag="nmax")
                    nc.vector.scalar_tensor_tensor(
                        nmax, maxTs[0], -1.0, nm1, ALU.mult, ALU.min
                    )
                    # e = exp(sT - max), fused per-block channel sums
                    eTs = []
                    accs = []
                    for mc in range(MC):
                        eT = epool.tile([P, NB, P], F16, tag="eT")
                        acc = accpool.tile([P, NB], F32, tag="acc")
                        for i in range(NB):
                            nc.scalar.activation(
                                out=eT[:, i, :], in_=sTs[mc][:, i, :],
                                func=AF.Exp, bias=nmax[:, i:i + 1], scale=1.0,
                                accum_out=acc[:, i:i + 1],
                            )
                        eTs.append(eT)
                        accs.append(acc)
                    sums = accpool.tile([P, NB], F32, tag="sums")
                    nc.vector.tensor_add(sums, accs[0], accs[1])
                    rT = accpool.tile([P, NB], F32, tag="rT")
                    nc.vector.reciprocal(rT, sums)
                    # renormalize + transpose back to channel-major
                    for mc in range(MC):
                        wT = epool.tile([P, NB, P], F16, tag="wT")
                        for i in range(NB):
                            nc.vector.tensor_scalar_mul(
                                wT[:, i, :], eTs[mc][:, i, :], rT[:, i:i + 1]
                            )
                        wb_ps = pT_pool.tile([P, NB, P], F16, tag="pT")
                        for i in range(NB):
                            nc.tensor.transpose(
                                wb_ps[:, i, :], wT[:, i, :], ident16
                            )
                        o_sb = opool.tile([P, NT], F16, tag="o")
                        nc.vector.tensor_mul(o_sb, wb_ps, v_sbs[(n, mc)])
                        nc.sync.dma_start(
                            out=y_out[b, mc, :, t0:t0 + NT], in_=o_sb
                        )
    nc.compile()
    return nc


def fold_weights(inputs: dict) -> dict:
    """Host-side folding of the tiny weight tensors into device layouts."""
    k_w = np.asarray(inputs["k_w"], np.float32)
    k_b = np.asarray(inputs["k_b"], np.float32)
    q_w = np.asarray(inputs["q_w"], np.float32)
    q_b = np.asarray(inputs["q_b"], np.float32)
    v_w = np.asarray(inputs["v_w"], np.float32)
    v_b = np.asarray(inputs["v_b"], np.float32)
    gamma = np.asarray(inputs["bn_gamma"], np.float32)
    beta = np.asarray(inputs["bn_beta"], np.float32)
    mean = np.asarray(inputs["bn_mean"], np.float32)
    var = np.asarray(inputs["bn_var"], np.float32)

    # kwT[p, l, kc, mc, m] = k_w[l, mc*128+m, kc*128+p]
    kwT = np.ascontiguousarray(
        k_w.reshape(L, MC, P, KC, P).transpose(4, 0, 3, 1, 2)
    ).astype(np.float16)
    kb = np.ascontiguousarray(k_b.reshape(L, MC, P)).astype(np.float16)
    qws = q_w.sum(axis=1)                       # [L, C]
    qwsT = np.ascontiguousarray(
        qws.reshape(L, KC, P).transpose(2, 1, 0)
    ).astype(np.float16)
    qbs = np.ascontiguousarray(q_b.sum(axis=1).reshape(L, 1))
    scale = gamma / np.sqrt(var + BN_EPS)
    vw_f = v_w * scale[:, None]
    vb_f = (v_b - mean) * scale + beta
    vwT = np.ascontiguousarray(
        vw_f.reshape(MC, P, KC, P).transpose(3, 2, 0, 1)
    ).astype(np.float16)
    vbT = np.ascontiguousarray(vb_f.reshape(MC, P).transpose(1, 0))
    return {
        "kwT": kwT, "kb": kb, "qwsT": qwsT, "qbs": qbs,
        "vwT": vwT, "vb": vbT,
        "ident": np.eye(P, dtype=np.float32),
        "ident16": np.eye(P, dtype=np.float16),
    }


_CACHE: dict = {}


def make_in_maps(inputs: dict) -> list:
    weights = fold_weights(inputs)
    x = np.asarray(inputs["x"], np.float32)
    # pad x on the left with zeros and convert to fp16
    xp = np.zeros((B, C, TP), np.float16)
    xp[:, :, PAD:] = x.astype(np.float16)
    xp = xp.reshape(B // BPC, BPC, KC, P, TP)
    return [
        {"x": np.ascontiguousarray(xp[i]), **weights} for i in range(NCORES)
    ]


def kernel(**inputs) -> np.ndarray:
    alpha = float(np.asarray(inputs["prelu_alpha"]).reshape(-1)[0])

    key = ("prog", alpha, PRELU_EXPLICIT)
    if key not in _CACHE:
        _CACHE[key] = build_program(alpha)
    nc = _CACHE[key]

    in_maps = make_in_maps(inputs)
    res = run_bass_kernel_spmd(nc, in_maps, list(range(NCORES)))
    outs = [r["y"].reshape(BPC, C, T).astype(np.float32) for r in res.results]
    return np.concatenate(outs, axis=0)


if __name__ == "__main__":
    rng = np.random.default_rng(0)
    demo = {
        "x": rng.standard_normal((B, C, T), dtype=np.float32),
        "q_w": rng.standard_normal((L, C, C), dtype=np.float32) / 16,
        "q_b": rng.standard_normal((L, C), dtype=np.float32) * 0.02,
        "k_w": rng.standard_normal((L, C, C), dtype=np.float32) / 16,
        "k_b": rng.standard_normal((L, C), dtype=np.float32) * 0.02,
        "v_w": rng.standard_normal((C, C), dtype=np.float32) / 16,
        "v_b": rng.standard_normal((C,), dtype=np.float32) * 0.02,
        "bn_gamma": rng.uniform(0.5, 1.5, C).astype(np.float32),
        "bn_beta": rng.standard_normal(C).astype(np.float32) * 0.02,
        "bn_mean": rng.standard_normal(C).astype(np.float32) * 0.1,
        "bn_var": rng.uniform(0.5, 1.5, C).astype(np.float32),
        "prelu_alpha": np.full((1,), 0.25, np.float32),
    }
    y = kernel(**demo)
    print("out", y.shape, y.dtype, float(np.abs(y).max()))


# revision 15
# speedup vs baseline: 1.9525x; 1.0183x over previous
"""Trainium2 Bass kernel for nn_ChannelAttention (B=16, C=256, T=2048, L=5).

Data-parallel over 8 NeuronCores: each core processes 2 batches.

Math (per batch b):
  qsum[l,t]   = qws[l] @ x[:,t] + qbs[l]                      (qws = q_w.sum(1))
  scores[c,t] = sum_l (k_w[l] @ (x * Bqsum[l]))[c, t-l] + sum_l k_b[l,c]*qsum[l,t-l]
  w = softmax_c(scores);  v = PReLU(BN(v_w @ x + v_b));  out = w * v

v2 design (fp16 datapath, validated 6.4e-3 rel err vs 2e-2 budget):
  - everything that feeds the PE is fp16 (1 cyc/row matmuls, FWL weight loads,
    half the DMA/SBUF traffic); PSUM accumulation stays fp32.
  - bq (qsum row broadcast over 128 partitions) via K=1 matmuls ROW-PACKED with
    tile_position at base partitions 0/32/64 (3+2 concurrent waves).
  - gating multiplies on DVE in fp16 (2x mode, ~330ns/tile); an x copy shifted
    by one element (x16o) keeps odd-lag windows 4B-aligned for 2x mode.
  - softmax over channels in the TRANSPOSED domain: PE-transpose scores,
    DVE free-dim max-reduce, ACT Exp with per-partition bias=-max and fused
    accum_out channel sums, tiny DVE reciprocal [128,4], per-partition
    tensor_scalar renormalize, PE-transpose back.  No ones-matmul broadcasts,
    no Ln (kills ACT table thrash), no [1,T]-row ops.
  - v phase grouped per batch so ACT Prelu<->Exp table switches happen at most
    twice per batch.
"""

import sys

sys.path.insert(0, "/opt/trn_rl_repo")

import numpy as np

import concourse.bass as bass
import concourse.mybir as mybir
import concourse.tile as tile
from concourse import bacc
from concourse.bass_utils import run_bass_kernel_spmd

B, C, T, L = 16, 256, 2048, 5
NCORES = 8
BPC = B // NCORES      # batches per core
P = 128                # partitions
KC = C // P            # k chunks (2)
MC = C // P            # m chunks (2)
NT = 512               # time tile
NB = NT // P           # transpose blocks per time tile (4)
NCHUNK = T // NT       # 4
PAD = 8                # left zero pad (t<0 lag windows)
TP = PAD + T           # padded time length
QPITCH = T + 16        # dram scratch row pitch for shifted qsum
BN_EPS = 1e-5

F32 = mybir.dt.float32
F32R = mybir.dt.float32r
F16 = mybir.dt.float16

# CoreSim lacks the Prelu activation: the sim path computes
# v = max(z, alpha*z) with two DVE ops instead.
PRELU_EXPLICIT = False

AF = mybir.ActivationFunctionType
ALU = mybir.AluOpType


def build_program(alpha: float) -> bass.Bass:
    nc = bacc.Bacc("TRN2", target_bir_lowering=False, debug=False, num_devices=NCORES)

    x_in = nc.dram_tensor("x", [BPC, KC, P, TP], F16, kind="ExternalInput").ap()
    kwT_in = nc.dram_tensor("kwT", [P, L, KC, MC, P], F16, kind="ExternalInput").ap()
    kb_in = nc.dram_tensor("kb", [L, MC, P], F16, kind="ExternalInput").ap()
    qwsT_in = nc.dram_tensor("qwsT", [P, KC, L], F16, kind="ExternalInput").ap()
    qbs_in = nc.dram_tensor("qbs", [L, 1], F32, kind="ExternalInput").ap()
    vwT_in = nc.dram_tensor("vwT", [P, KC, MC, P], F16, kind="ExternalInput").ap()
    vb_in = nc.dram_tensor("vb", [P, MC], F32, kind="ExternalInput").ap()
    ident_in = nc.dram_tensor("ident", [P, P], F32R, kind="ExternalInput").ap()
    ident16_in = nc.dram_tensor("ident16", [P, P], F16, kind="ExternalInput").ap()
    y_out = nc.dram_tensor("y", [BPC, MC, P, T], F16, kind="ExternalOutput").ap()
    # scratch for the lag-shift of qsum rows (row l shifted right by l)
    qsd = nc.dram_tensor("qs_scratch", [BPC, L, QPITCH], F16).ap()

    from contextlib import ExitStack

    with tile.TileContext(nc) as tc:
        with ExitStack() as ctx:
            ep = ctx.enter_context
            ep(nc.allow_low_precision(
                reason="fp16 datapath validated at 6.4e-3 rel err vs the "
                       "2e-2 budget; PSUM accumulation stays fp32"
            ))
            consts = ep(tc.tile_pool(name="consts", bufs=1))
            xpool = ep(tc.tile_pool(name="xpool", bufs=2))
            qspool = ep(tc.tile_pool(name="qspool", bufs=2))
            qsshpool = ep(tc.tile_pool(name="qsshpool", bufs=2))
            bqlpool = ep(tc.tile_pool(name="bqlpool", bufs=10))
            vpool = ep(tc.tile_pool(name="vpool", bufs=12))
            wpool = ep(tc.tile_pool(name="wpool", bufs=12))
            spool = ep(tc.tile_pool(name="spool", bufs=4))
            epool = ep(tc.tile_pool(name="epool", bufs=6))
            accpool = ep(tc.tile_pool(name="accpool", bufs=12))
            opool = ep(tc.tile_pool(name="opool", bufs=4))
            # PSUM: 8 banks.  pscore 2 + pbq 2 + pT 3 + pv 1.
            pscore_pool = ep(tc.tile_pool(name="pscore", bufs=2, space="PSUM"))
            pbq_pool = ep(tc.tile_pool(name="pbq", bufs=2, space="PSUM"))
            pT_pool = ep(tc.tile_pool(name="pT", bufs=3, space="PSUM"))
            pv_pool = ep(tc.tile_pool(name="pv", bufs=1, space="PSUM"))

            # ---- constants ----
            kwT = consts.tile([P, L, KC, MC, P], F16)
            nc.sync.dma_start(out=kwT, in_=kwT_in)
            kb = consts.tile([L, MC, P], F16)
            nc.sync.dma_start(out=kb, in_=kb_in)
            qwsT = consts.tile([P, KC, L], F16)
            nc.sync.dma_start(out=qwsT, in_=qwsT_in)
            qbs = consts.tile([L, 1], F32)
            nc.sync.dma_start(out=qbs, in_=qbs_in)
            vwT = consts.tile([P, KC, MC, P], F16)
            nc.sync.dma_start(out=vwT, in_=vwT_in)
            vb = consts.tile([P, MC], F32)
            nc.sync.dma_start(out=vb, in_=vb_in)
            ident = consts.tile([P, P], F32R)        # PE transpose (fp32r scores)
            nc.sync.dma_start(out=ident, in_=ident_in)
            ident16 = consts.tile([P, P], F16)       # PE transpose (fp16 weights)
            nc.sync.dma_start(out=ident16, in_=ident16_in)
            zpad = consts.tile([L, PAD], F16)        # zero left pad for qsd
            nc.vector.memset(zpad, 0.0)


            for b in range(BPC):
                # ---- load x: one tile per kc (clean 2-dim APs for DVE
                # perf modes), quarter-split so qsum starts early ----
                x_sbs = [xpool.tile([P, TP], F16, tag=f"x{kc}", name=f"x{kc}")
                         for kc in range(KC)]
                for q in range(4):
                    a0 = q * (TP // 4)
                    a1 = TP if q == 3 else (q + 1) * (TP // 4)
                    for kc in range(KC):
                        nc.sync.dma_start(
                            out=x_sbs[kc][:, a0:a1], in_=x_in[b, kc, :, a0:a1]
                        )

                # ---- qsum rows: qs[l,t] = qws[l] @ x[:,t] + qbs[l] ----
                qs_sb = qspool.tile([L, T], F16, tag="qs")
                for n in range(NCHUNK):
                    qs_ps = pbq_pool.tile([L, NT], F32, tag="pbq")
                    for kc in range(KC):
                        nc.tensor.matmul(
                            qs_ps,
                            qwsT[:, kc, :],
                            x_sbs[kc][:, PAD + n * NT:PAD + (n + 1) * NT],
                            start=(kc == 0),
                            stop=(kc == KC - 1),
                        )
                    nc.vector.tensor_scalar_add(
                        qs_sb[:, n * NT:(n + 1) * NT], qs_ps, qbs
                    )
                    # stream the rows to dram as they are produced
                    nc.sync.dma_start(
                        out=qsd[b, :, PAD + n * NT:PAD + (n + 1) * NT],
                        in_=qs_sb[:, n * NT:(n + 1) * NT],
                    )
                nc.sync.dma_start(out=qsd[b, :, 0:PAD], in_=zpad)
                nc.sync.dma_start(out=qsd[b, :, PAD + T:QPITCH], in_=zpad)

                # xo[j] = x[j+1] (even element offsets for odd lags); loaded
                # after the qsum phase so it stays off the startup path
                xo_sbs = [xpool.tile([P, TP], F16, tag=f"xo{kc}", name=f"xo{kc}")
                          for kc in range(KC)]
                for kc in range(KC):
                    nc.sync.dma_start(
                        out=xo_sbs[kc][:, 0:TP - 1], in_=x_in[b, kc, :, 1:TP]
                    )

                # ---- shifted qsum views via DRAM round trip (half-split so
                # the first chunks unblock early) ----
                qssh_sb = qsshpool.tile([L, T], F16, tag="qssh")
                for hf in range(2):
                    h0 = hf * (T // 2)
                    shifted = bass.AP(
                        tensor=qsd.tensor,
                        offset=b * L * QPITCH + PAD + h0,
                        ap=[[QPITCH - 1, L], [1, T // 2]],
                    )
                    nc.sync.dma_start(
                        out=qssh_sb[:, h0:h0 + T // 2], in_=shifted
                    )
                # bql[l][p, j] = qsum[l, j-8-l]: the lag-shifted qsum row
                # broadcast to all 128 partitions via a stride-0-partition DMA.
                # (cols j<8 read the previous row's tail; never used.)
                bqls = []
                for l in range(L):
                    bql = bqlpool.tile([P, TP], F16, tag="bql")
                    for hf in range(2):
                        h0 = hf * (TP // 2)
                        h1 = TP - h0 - (TP // 2)
                        bcast = bass.AP(
                            tensor=qsd.tensor,
                            offset=(b * L + l) * QPITCH - l + h0,
                            ap=[[0, P], [1, TP - h0 if hf else TP // 2]],
                        )
                        nc.sync.dma_start(out=bql[:, h0:TP if hf else h0 + TP // 2],
                                          in_=bcast)
                    bqls.append(bql)

                # ---- v phase (grouped: one Prelu table window per batch) ----
                v_sbs = {}
                for n in range(NCHUNK):
                    t0 = n * NT
                    for mc in range(MC):
                        v_ps = pv_pool.tile([P, NT], F32, tag="pv")
                        for kc in range(KC):
                            nc.tensor.matmul(
                                v_ps,
                                vwT[:, kc, mc, :],
                                x_sbs[kc][:, PAD + t0:PAD + t0 + NT],
                                start=(kc == 0),
                                stop=(kc == KC - 1),
                            )
                        v_sb = vpool.tile([P, NT], F16, tag="v")
                        if PRELU_EXPLICIT:
                            vz = spool.tile([P, NT], F32, tag="vz")
                            nc.vector.tensor_scalar_add(vz, v_ps, vb[:, mc:mc + 1])
                            nc.vector.scalar_tensor_tensor(
                                v_sb, vz, float(alpha), vz, ALU.mult, ALU.max
                            )
                        else:
                            nc.scalar.activation(
                                out=v_sb, in_=v_ps, func=AF.Prelu,
                                bias=vb[:, mc:mc + 1], scale=1.0, alpha=alpha,
                            )
                        v_sbs[(n, mc)] = v_sb

                # ---- time-chunk loop ----
                for n in range(NCHUNK):
                    t0 = n * NT
                    s0 = PAD + t0

                    # gated tiles: w_l[:, i] = x[:, s0-l+i] * qsum[l, t0-l+i]
                    w_tiles = []
                    for l in range(L):
                        for kc in range(KC):
                            w_sb = wpool.tile([P, NT], F16, tag="w")
                            if l % 2 == 0:
                                xa = x_sbs[kc][:, s0 - l:s0 - l + NT]
                            else:
                                xa = xo_sbs[kc][:, s0 - l - 1:s0 - l - 1 + NT]
                            bq = bqls[l][:, s0:s0 + NT]
                            if l >= 3:
                                nc.gpsimd.tensor_mul(w_sb, xa, bq)
                            else:
                                nc.vector.tensor_mul(w_sb, xa, bq)
                            w_tiles.append(w_sb)

                    # scores: ps[mc] = sum_{l,kc} kwT[l,kc,mc].T @ w[l,kc]
                    #         + kb[:,mc].T @ qssh[:, t0:t0+NT]
                    pscores = []
                    for mc in range(MC):
                        ps = pscore_pool.tile([P, NT], F32, tag="ps")
                        for l in range(L):
                            for kc in range(KC):
                                nc.tensor.matmul(
                                    ps,
                                    kwT[:, l, kc, mc, :],
                                    w_tiles[l * KC + kc],
                                    start=(l == 0 and kc == 0),
                                    stop=False,
                                )
                        nc.tensor.matmul(
                            ps,
                            kb[:, mc, :],
                            qssh_sb[:, t0:t0 + NT],
                            start=False, stop=True,
                        )
                        pscores.append(ps)

                    # ---- softmax over channels in the transposed domain ----
                    s_sbs = []
                    sTs = []
                    for mc in range(MC):
                        s_sb = spool.tile([P, NT], F32R, tag="s")
                        nc.scalar.copy(out=s_sb, in_=pscores[mc])
                        s_sbs.append(s_sb)
                        sT = pT_pool.tile([P, NB, P], F32R, tag="pT")
                        for i in range(NB):
                            nc.tensor.transpose(
                                sT[:, i, :], s_sb[:, i * P:(i + 1) * P], ident
                            )
                        sTs.append(sT.bitcast(F32))
                    maxTs = []
                    for mc in range(MC):
                        maxT = accpool.tile([P, NB], F32, tag="maxT")
                        nc.vector.tensor_reduce(
                            out=maxT, in_=sTs[mc], axis=mybir.AxisListType.X,
                            op=ALU.max,
                        )
                        maxTs.append(maxT)
                    # one COARSE max per transposed partition (max over all 4
                    # column blocks): softmax is invariant to the subtracted
                    # value, and the cross-block spread is far below fp16
                    # underflow, so a single [P,1] bias serves the whole tile.
                    # nmax[p,i] = -max over both channel halves (per-block
                    # biases: every block sum then contains e^0 = 1, so the
                    # fp16 eT tiles can never underflow to an all-zero row)
                    mx = accpool.tile([P, NB], F32, tag="mx")
                    nc.vector.tensor_max(mx, maxTs[0], maxTs[1])
                    nmax = accpool.tile([P, NB], F32, tag="nmax")
                    nc.vector.tensor_scalar_mul(nmax, mx, -1.0)
                    # e = exp(sT - max), channel sums via DVE reduce
                    eTs = []
                    accs = []
                    for mc in range(MC):
                        eT = epool.tile([P, NB, P], F16, tag="eT")
                        for i in range(NB):
                            nc.scalar.activation(
                                out=eT[:, i, :], in_=sTs[mc][:, i, :],
                                func=AF.Exp, bias=nmax[:, i:i + 1], scale=1.0,
                            )
                        acc = accpool.tile([P, NB], F32, tag="acc")
                        nc.vector.tensor_reduce(
                            out=acc, in_=eT, axis=mybir.AxisListType.X,
                            op=ALU.add,
                        )
                        eTs.append(eT)
                        accs.append(acc)
                    sums = accpool.tile([P, NB], F32, tag="sums")
                    nc.vector.tensor_add(sums, accs[0], accs[1])
                    rT = accpool.tile([P, NB], F32, tag="rT")
                    nc.vector.reciprocal(rT, sums)
                    # renormalize + transpose back to channel-major
                    for mc in range(MC):
                        wT = epool.tile([P, NB, P], F16, tag="wT")
                        for i in range(NB):
                            if mc == 0:
                                nc.vector.tensor_scalar_mul(
                                    wT[:, i, :], eTs[mc][:, i, :],
                                    rT[:, i:i + 1],
                                )
                            else:
                                nc.scalar.activation(
                                    out=wT[:, i, :], in_=eTs[mc][:, i, :],
                                    func=AF.Copy, scale=rT[:, i:i + 1],
                                )
                        wb_ps = pT_pool.tile([P, NB, P], F16, tag="pT")
                        for i in range(NB):
                            nc.tensor.transpose(
                                wb_ps[:, i, :], wT[:, i, :], ident16
                            )
                        o_sb = opool.tile([P, NT], F16, tag="o")
                        nc.vector.tensor_mul(o_sb, wb_ps, v_sbs[(n, mc)])
                        nc.sync.dma_start(
                            out=y_out[b, mc, :, t0:t0 + NT], in_=o_sb
                        )
    nc.compile()
    return nc


def fold_weights(inputs: dict) -> dict:
    """Host-side folding of the tiny weight tensors into device layouts."""
    k_w = np.asarray(inputs["k_w"], np.float32)
    k_b = np.asarray(inputs["k_b"], np.float32)
    q_w = np.asarray(inputs["q_w"], np.float32)
    q_b = np.asarray(inputs["q_b"], np.float32)
    v_w = np.asarray(inputs["v_w"], np.float32)
    v_b = np.asarray(inputs["v_b"], np.float32)
    gamma = np.asarray(inputs["bn_gamma"], np.float32)
    beta = np.asarray(inputs["bn_beta"], np.float32)
    mean = np.asarray(inputs["bn_mean"], np.float32)
    var = np.asarray(inputs["bn_var"], np.float32)

    # kwT[p, l, kc, mc, m] = k_w[l, mc*128+m, kc*128+p]
    kwT = np.ascontiguousarray(
        k_w.reshape(L, MC, P, KC, P).transpose(4, 0, 3, 1, 2)
    ).astype(np.float16)
    kb = np.ascontiguousarray(k_b.reshape(L, MC, P)).astype(np.float16)
    qws = q_w.sum(axis=1)                       # [L, C]
    qwsT = np.ascontiguousarray(
        qws.reshape(L, KC, P).transpose(2, 1, 0)
    ).astype(np.float16)
    qbs = np.ascontiguousarray(q_b.sum(axis=1).reshape(L, 1))
    scale = gamma / np.sqrt(var + BN_EPS)
    vw_f = v_w * scale[:, None]
    vb_f = (v_b - mean) * scale + beta
    vwT = np.ascontiguousarray(
        vw_f.reshape(MC, P, KC, P).transpose(3, 2, 0, 1)
    ).astype(np.float16)
    vbT = np.ascontiguousarray(vb_f.reshape(MC, P).transpose(1, 0))
    return {
        "kwT": kwT, "kb": kb, "qwsT": qwsT, "qbs": qbs,
        "vwT": vwT, "vb": vbT,
        "ident": np.eye(P, dtype=np.float32),
        "ident16": np.eye(P, dtype=np.float16),
    }


_CACHE: dict = {}


def make_in_maps(inputs: dict) -> list:
    weights = fold_weights(inputs)
    x = np.asarray(inputs["x"], np.float32)
    # pad x on the left with zeros and convert to fp16
    xp = np.zeros((B, C, TP), np.float16)
    xp[:, :, PAD:] = x.astype(np.float16)
    xp = xp.reshape(B // BPC, BPC, KC, P, TP)
    return [
        {"x": np.ascontiguousarray(xp[i]), **weights} for i in range(NCORES)
    ]


def kernel(**inputs) -> np.ndarray:
    alpha = float(np.asarray(inputs["prelu_alpha"]).reshape(-1)[0])

    key = ("prog", alpha, PRELU_EXPLICIT)
    if key not in _CACHE:
        _CACHE[key] = build_program(alpha)
    nc = _CACHE[key]

    in_maps = make_in_maps(inputs)
    res = run_bass_kernel_spmd(nc, in_maps, list(range(NCORES)))
    outs = [r["y"].reshape(BPC, C, T).astype(np.float32) for r in res.results]
    return np.concatenate(outs, axis=0)


if __name__ == "__main__":
    rng = np.random.default_rng(0)
    demo = {
        "x": rng.standard_normal((B, C, T), dtype=np.float32),
        "q_w": rng.standard_normal((L, C, C), dtype=np.float32) / 16,
        "q_b": rng.standard_normal((L, C), dtype=np.float32) * 0.02,
        "k_w": rng.standard_normal((L, C, C), dtype=np.float32) / 16,
        "k_b": rng.standard_normal((L, C), dtype=np.float32) * 0.02,
        "v_w": rng.standard_normal((C, C), dtype=np.float32) / 16,
        "v_b": rng.standard_normal((C,), dtype=np.float32) * 0.02,
        "bn_gamma": rng.uniform(0.5, 1.5, C).astype(np.float32),
        "bn_beta": rng.standard_normal(C).astype(np.float32) * 0.02,
        "bn_mean": rng.standard_normal(C).astype(np.float32) * 0.1,
        "bn_var": rng.uniform(0.5, 1.5, C).astype(np.float32),
        "prelu_alpha": np.full((1,), 0.25, np.float32),
    }
    y = kernel(**demo)
    print("out", y.shape, y.dtype, float(np.abs(y).max()))
